# revision 16
# baseline (speedup 1.0000x reference)
"""Trainium2 Bass kernel for nn_DenseExpert (MoE dense-expert gated blend).

Math (full problem, B=8192, E=8, U=512, D=512):
    h[b,e,u] = sum_d x[b,d] * alpha[e,u,d]
    r[b,u]   = sum_e g[b,e] * h[b,e,u] + sum_e g[b,e] * beta[e,u]

Strategy:
  - Data-parallel over batch B across 8 NeuronCores (1024 rows each);
    alpha/beta replicated. No collectives.
  - With all 8 cores streaming a replicated 8 MB alpha, the kernel is
    paced by aggregate HBM bandwidth, so the matmul operands (x, alpha)
    are converted to bf16 on the host — halves the dominant traffic and
    keeps the PE at full rate (1 cycle/row). Measured scale-relative
    error ~2e-3. The bias matmul operands are bf16 too (bias is ~5% of the output scale; PSUM accumulation stays fp32).
  - DMA issue on the queue engine costs ~0.65us per dma_start, so
    transfers are consolidated: one DMA for [g^T|beta], one for g, one
    for all of x^T, one per expert for alpha^T (3D access patterns
    place the contraction dim d on SBUF partitions), one for the
    output.
  - Per core: for each expert e, h_e = x @ alpha[e]^T as 4 accumulating
    bf16 matmuls per 128-row batch tile, k-outer (8 PSUM banks). The
    gated blend is one fused DVE op per (expert, tile):
    acc = psum * g[:,e] + acc. The bias sum_e g[b,e]*beta[e,u] is one
    K=8 matmul per tile, folded into expert 0's blend.
  - The batch tiles are processed in three phases (m0-4, m5-6, m7),
    each running the full expert loop, so the output slab of a finished
    phase DMAs back to HBM while the next phase computes — only the
    last 0.25 MB write remains on the tail.
"""

import numpy as np
from contextlib import ExitStack

try:
    import concourse.bass as bass
except ImportError:  # fallback if concourse isn't on the default path
    import sys

    sys.path.insert(0, "/opt/trn_rl_repo")
    import concourse.bass as bass
from concourse import bacc

import concourse.mybir as mybir
import concourse.tile as tile
from concourse.bass_utils import run_bass_kernel_spmd

B, E, U, D = 8192, 8, 512, 512
N_CORES = 8
BC = B // N_CORES  # 1024 batch rows per core
P = 128
M_TILES = BC // P  # 8 batch tiles per core
K_TILES = D // P  # 4 contraction chunks
F32 = mybir.dt.float32
F32R = mybir.dt.float32r
BF16 = mybir.dt.bfloat16

_NC_CACHE = {}
last_results = None  # BassKernelResults of the most recent run (for test harness)


def _build_nc():
    nc = bacc.Bacc("TRN2", target_bir_lowering=False, debug=False)

    xT = nc.dram_tensor("xT", [D, BC], BF16, kind="ExternalInput").ap()
    g = nc.dram_tensor("g", [BC, E], F32, kind="ExternalInput").ap()
    # gb = [g^T | beta] packed so one DMA covers both bias-matmul operands
    gb = nc.dram_tensor("gb", [E, BC + U], BF16, kind="ExternalInput").ap()
    alphaT = nc.dram_tensor("alphaT", [E, D, U], BF16, kind="ExternalInput").ap()
    out = nc.dram_tensor("out", [BC, U], F32, kind="ExternalOutput").ap()

    mult = mybir.AluOpType.mult
    add = mybir.AluOpType.add

    with tile.TileContext(nc) as tc, ExitStack() as ctx:
        sml_pool = ctx.enter_context(tc.tile_pool(name="sml", bufs=1))
        at_pool = ctx.enter_context(tc.tile_pool(name="at", bufs=E))
        acc_pool = ctx.enter_context(tc.tile_pool(name="acc", bufs=1))
        ps_pool = ctx.enter_context(tc.tile_pool(name="ps", bufs=8, space="PSUM"))

        # ---- DMA priority order: bias operands, g, x, then experts ----
        gb_t = sml_pool.tile([E, BC + U], BF16, tag="gb", name="gb")
        nc.sync.dma_start(gb_t[:], gb[:, :])

        # all of g in one DMA: [128, m, e]
        g_t = sml_pool.tile([P, M_TILES, E], F32, tag="g", name="gt")
        nc.sync.dma_start(g_t[:], g.rearrange("(m p) e -> p m e", p=P))

        # all of x^T in one DMA: [128, k, b];  slot (p, k) holds d = k*128+p
        xt_t = sml_pool.tile([P, K_TILES, BC], BF16, tag="xt", name="xt")
        nc.sync.dma_start(xt_t[:], xT.rearrange("(k p) b -> p k b", p=P))

        # alpha^T: one DMA per expert: [128, k, u]; slot (p, k) holds d = k*128+p
        ats = []
        for e in range(E):
            a_t = at_pool.tile([P, K_TILES, U], BF16, tag="at", name=f"at{e}")
            nc.sync.dma_start(a_t[:], alphaT[e].rearrange("(k p) u -> p k u", p=P))
            ats.append(a_t)

        # ---- bias: b_m = g @ beta (K=8 fp32r matmuls, copied to SBUF) ----
        bias_t = acc_pool.tile([P, M_TILES, U], F32, tag="bias", name="bias")
        for m in range(M_TILES):
            pb_t = ps_pool.tile([P, U], F32, tag="ps", name=f"pb{m}")
            nc.tensor.matmul(
                pb_t[:],
                gb_t[:, m * P : (m + 1) * P],
                gb_t[:, BC : BC + U],
                start=True,
                stop=True,
            )
            nc.vector.tensor_copy(bias_t[:, m, :], pb_t[:])

        # ---- experts, phased over batch tiles so output writes overlap
        # compute: full expert loop for m0-4, write that slab while
        # computing m5-6, then m7 (small tail) ----
        acc_t = acc_pool.tile([P, M_TILES, U], F32, tag="acc", name="acc")
        out_r = out.rearrange("(m p) u -> p m u", p=P)
        PHASES = [(0, 5), (5, 7), (7, 8)]
        for a, b in PHASES:
            for e in range(E):
                pes = {}
                for m in range(a, b):
                    pes[m] = ps_pool.tile([P, U], F32, tag="ps", name=f"pe{e}_{m}")
                for k in range(K_TILES):
                    for m in range(a, b):
                        nc.tensor.matmul(
                            pes[m][:],
                            xt_t[:, k, bass.ts(m, P)],
                            ats[e][:, k, :],
                            start=(k == 0),
                            stop=(k == K_TILES - 1),
                        )
                for m in range(a, b):
                    gcol = g_t[:, m, e : e + 1]
                    if e == 0:
                        # acc = h_0 * g[:,0] + bias
                        nc.vector.scalar_tensor_tensor(
                            acc_t[:, m, :], pes[m][:], gcol, bias_t[:, m, :],
                            op0=mult, op1=add,
                        )
                    else:
                        # acc += h_e * g[:,e]
                        nc.vector.scalar_tensor_tensor(
                            acc_t[:, m, :], pes[m][:], gcol, acc_t[:, m, :],
                            op0=mult, op1=add,
                        )
            nc.sync.dma_start(out_r[:, a:b, :], acc_t[:, a:b, :])

    nc.compile()
    return nc


def _get_nc():
    if "nc" not in _NC_CACHE:
        _NC_CACHE["nc"] = _build_nc()
    return _NC_CACHE["nc"]


def kernel(x, g, alpha, beta, _trace=False, _trace_kwargs=None):
    global last_results
    import ml_dtypes

    bf16 = ml_dtypes.bfloat16
    x = np.asarray(x, dtype=np.float32)
    g = np.ascontiguousarray(np.asarray(g, dtype=np.float32))
    alpha = np.asarray(alpha, dtype=np.float32)
    beta = np.ascontiguousarray(np.asarray(beta, dtype=np.float32))

    # [E, D, U] in bf16 for halved DMA traffic
    alphaT = np.ascontiguousarray(alpha.transpose(0, 2, 1).astype(bf16))
    xTb = np.ascontiguousarray(x.T.astype(bf16))  # [D, B]

    in_maps = []
    for c in range(N_CORES):
        sl = slice(c * BC, (c + 1) * BC)
        gc = g[sl]
        in_maps.append(
            {
                "xT": np.ascontiguousarray(xTb[:, sl]),  # [D, BC] bf16
                "g": gc,  # [BC, E] f32
                "gb": np.ascontiguousarray(
                    np.concatenate([gc.T, beta], axis=1).astype(bf16)
                ),  # [E, BC + U] bf16
                "alphaT": alphaT,  # [E, D, U] bf16 (replicated)
            }
        )

    nc = _get_nc()
    res = run_bass_kernel_spmd(
        nc,
        in_maps,
        list(range(N_CORES)),
        trace=_trace,
        **(_trace_kwargs or {}),
    )
    last_results = res
    return np.concatenate([r["out"] for r in res.results], axis=0)
